# revision 24
# baseline (speedup 1.0000x reference)
"""Trainium2 Bass kernel for Jaccard cosine-similarity edge masking.

out[e] = edge_weight[e] * (sim(e) >= 0.01) * (1 + (src==dst)),
sim(e) = <f_src, f_dst> / (||f_src|| * ||f_dst|| + 1e-8)

Three-stage device pipeline, edges sharded across 8 NeuronCores:

  NEFF1 (norm):   node table row-sharded 8 ways; each core computes
                  ||f|| per row (squares on the ACT engine, reduce on
                  DVE, all fp32) and emits an fp16 copy of its feature
                  shard via cast-during-DMA (SWDGE), plus fp32 and fp16
                  norm vectors.
  NEFF2 (edge):   per-edge inner products over host-gathered fp16 rows
                  (gather is pure indexing), streamed as ~1MiB linear
                  DMAs in a transposed [dim=partition, edge=free]
                  layout.  DVE does ONE fp16 multiply pass; the
                  per-edge reduction runs on the idle TensorEngine:
                  each 128-edge product tile is the matmul stationary
                  against a ones[128,1] moving vector, writing one PSUM
                  column of 128 distinct fp32 sums (~38ns/matmul); 512
                  matmuls fill a PSUM bank, drained by one DVE copy
                  straight into the edge-major [P,T] inner matrix.
                  Threshold test against q = thr*(ns*nd+eps) with
                  device-computed norms (fp16 stream); emits wout and
                  an ambiguity flag |inner - q| < q*(DELTA/thr),
                  finalized per 512-column block to overlap the tail.
  NEFF3 (rescue): flagged edges (~0.7%) recomputed exactly in fp32
                  from the original rows + device fp32 norms, making
                  the final output match the fp32 reference exactly
                  (fp16 noise is <~1.1e-4 in sim units, DELTA=8e-4).

If the edge list is detected (host-side comparison only) to be the
symmetric duplication [[s,d],[d,s]] with tied weights, only the first
half is computed and mirrored.

Host-side work is strictly indexing/layout: gathers of device-produced
tables, reshapes, and np.flatnonzero on a device-produced flag.  (This
environment's neuronxcc lowering miscompiles descriptor-based device
gather primitives, so row gathers are host-side.)
"""

import numpy as np
from contextlib import ExitStack

import concourse.bass as bass
import concourse.tile as tile
from concourse import bacc, mybir
from concourse.bass_utils import run_bass_kernel_spmd

N_NODES = 100000
N_EDGES = 1600000
D = 128
P = 128
N_CORES = 8
THRESHOLD = 0.01
EPS = 1e-8
DELTA = 8e-4          # ambiguity window in sim units (~3.5x max fp16 noise)

# NEFF1 geometry: 12500-row shard -> 98 tiles of 128 rows (last overlaps)
NPC = N_NODES // N_CORES          # 12500
NT = (NPC + P - 1) // P           # 98
LAST_ROW0 = NPC - P               # 12372
G1 = 14                           # tiles per load group
NG1 = NT // G1                    # 7

# NEFF2 geometry
M = 32                            # 128-edge tiles per load group (1 MiB fp16 DMA)

# NEFF3 geometry
MR = 16                           # rescue tiles (2048 edges/core)
MRC = 8                           # tiles per pipelined chunk
RSLOTS = MR * P                   # 2048
R_TOTAL = RSLOTS * N_CORES        # 16384

_cache = {}


def _build_norm_nc():
    """NEFF1: fp32 norms + fp16 table copy of a 12500-row shard."""
    nc = bacc.Bacc("TRN2", target_bir_lowering=False, debug=False,
                   num_devices=N_CORES)
    f32, f16 = mybir.dt.float32, mybir.dt.float16
    feat = nc.dram_tensor("feat_sw", [NG1, P, G1, D], f32, kind="ExternalInput")
    u16 = nc.dram_tensor("u16_sw", [NG1, P, G1, D], f16, kind="ExternalOutput")
    norm = nc.dram_tensor("norm98", [P, NT], f32, kind="ExternalOutput")
    norm16 = nc.dram_tensor("norm98h", [P, NT], f16, kind="ExternalOutput")
    with tile.TileContext(nc) as tc, ExitStack() as ctx:
        loads = ctx.enter_context(tc.tile_pool(name="loads", bufs=5))
        scr = ctx.enter_context(tc.tile_pool(name="scr", bufs=3))
        acc = ctx.enter_context(tc.tile_pool(name="acc", bufs=1))
        ss = acc.tile([P, NT], f32)
        for g in range(NG1):
            x = loads.tile([P, G1, D], f32, tag="x")
            eng = nc.sync if g % 2 == 0 else nc.scalar
            eng.dma_start(out=x[:], in_=feat.ap()[g])
            prod = scr.tile([P, G1, D], f32, tag="prod")
            # squares on the ACT engine (keeps DVE free for the reduce)
            nc.scalar.square(out=prod[:], in_=x[:])
            nc.vector.tensor_reduce(out=ss[:, g * G1:(g + 1) * G1],
                                    in_=prod[:],
                                    axis=mybir.AxisListType.X,
                                    op=mybir.AluOpType.add)
            # fp32 -> fp16 cast during DMA (SWDGE)
            nc.gpsimd.dma_start(out=u16.ap()[g], in_=x[:])
        nrm = acc.tile([P, NT], f32)
        nrm16 = acc.tile([P, NT], f16)
        nc.scalar.sqrt(out=nrm[:], in_=ss[:])
        nc.vector.tensor_copy(out=nrm16[:], in_=nrm[:])
        nc.sync.dma_start(out=norm.ap(), in_=nrm[:])
        nc.sync.dma_start(out=norm16.ap(), in_=nrm16[:])
    nc.compile()
    return nc


def _edge_geometry(epc):
    t = ((epc + P - 1) // P + M - 1) // M * M
    return t, t * P


GROUP_E = M * P                   # 4096 edges per load group
PBLK = 512                        # PSUM bank columns (f32)


def _build_edge_nc(epc):
    """NEFF2: fp16 products on DVE (transposed layout: partition dim =
    feature dim), per-128-edge sums via TensorE matmul with the edge tile
    as stationary and a ones[128,1] moving vector -> one PSUM column of
    128 distinct per-edge fp32 sums.  512 matmuls fill a [128,512] PSUM
    bank, drained with one DVE copy into the edge-major [P,T] inner
    matrix.  Threshold mask + ambiguity flag as before."""
    T, SLOTS = _edge_geometry(epc)
    G = T // M
    nc = bacc.Bacc("TRN2", target_bir_lowering=False, debug=False,
                   num_devices=N_CORES)
    f32, f16, i32 = mybir.dt.float32, mybir.dt.float16, mybir.dt.int32
    us = nc.dram_tensor("us", [G, P, GROUP_E], f16, kind="ExternalInput")
    ud = nc.dram_tensor("ud", [G, P, GROUP_E], f16, kind="ExternalInput")
    w_m = nc.dram_tensor("w_m", [P, T], f32, kind="ExternalInput")
    ns_m = nc.dram_tensor("ns_m", [P, T], f16, kind="ExternalInput")
    nd_m = nc.dram_tensor("nd_m", [P, T], f16, kind="ExternalInput")
    src_m = nc.dram_tensor("src_m", [P, T], i32, kind="ExternalInput")
    dst_m = nc.dram_tensor("dst_m", [P, T], i32, kind="ExternalInput")
    wout = nc.dram_tensor("wout", [P, T], f32, kind="ExternalOutput")
    amb = nc.dram_tensor("amb", [P, T], f16, kind="ExternalOutput")

    with tile.TileContext(nc) as tc, ExitStack() as ctx:
        mats = ctx.enter_context(tc.tile_pool(name="mats", bufs=1))
        loads = ctx.enter_context(tc.tile_pool(name="loads", bufs=4))
        scr = ctx.enter_context(tc.tile_pool(name="scr", bufs=3))
        psum = ctx.enter_context(tc.psum_pool(name="ps", bufs=2))

        w_s = mats.tile([P, T], f32)
        ns_s = mats.tile([P, T], f16)
        nd_s = mats.tile([P, T], f16)
        src_s = mats.tile([P, T], i32)
        dst_s = mats.tile([P, T], i32)
        inner = mats.tile([P, T], f32)
        ones = mats.tile([P, 1], f16)
        nc.vector.memset(ones[:], 1.0)
        nc.gpsimd.dma_start(out=w_s[:], in_=w_m.ap())
        nc.gpsimd.dma_start(out=ns_s[:], in_=ns_m.ap())
        nc.gpsimd.dma_start(out=nd_s[:], in_=nd_m.ap())
        nc.gpsimd.dma_start(out=src_s[:], in_=src_m.ap())
        nc.gpsimd.dma_start(out=dst_s[:], in_=dst_m.ap())

        q = mats.tile([P, T], f32)
        keep = mats.tile([P, T], f32)
        eq = mats.tile([P, T], f32)
        wo = mats.tile([P, T], f32)
        mg = mats.tile([P, T], f32)
        af = mats.tile([P, T], f16)

        def finals(c0, c1):
            """Threshold mask + ambiguity flag + output DMA for columns
            [c0, c1) -- called per drained PSUM block so the tail work
            overlaps the remaining stream."""
            s_ = (slice(None), slice(c0, c1))
            nc.vector.tensor_mul(out=q[s_], in0=ns_s[s_], in1=nd_s[s_])
            nc.vector.tensor_scalar(out=q[s_], in0=q[s_],
                                    scalar1=float(EPS),
                                    scalar2=float(THRESHOLD),
                                    op0=mybir.AluOpType.add,
                                    op1=mybir.AluOpType.mult)
            nc.vector.tensor_tensor(out=keep[s_], in0=inner[s_], in1=q[s_],
                                    op=mybir.AluOpType.is_ge)
            nc.vector.tensor_tensor(out=eq[s_], in0=src_s[s_], in1=dst_s[s_],
                                    op=mybir.AluOpType.is_equal)
            nc.vector.tensor_scalar(out=eq[s_], in0=eq[s_],
                                    scalar1=1.0, scalar2=None,
                                    op0=mybir.AluOpType.add)
            nc.vector.tensor_mul(out=wo[s_], in0=w_s[s_], in1=keep[s_])
            nc.vector.tensor_mul(out=wo[s_], in0=wo[s_], in1=eq[s_])
            # |inner - q| < q*(DELTA/thr)  ->  rescue flag
            nc.vector.tensor_tensor(out=mg[s_], in0=inner[s_], in1=q[s_],
                                    op=mybir.AluOpType.subtract)
            nc.scalar.activation(out=mg[s_], in_=mg[s_],
                                 func=mybir.ActivationFunctionType.Abs)
            nc.vector.tensor_scalar(out=q[s_], in0=q[s_],
                                    scalar1=float(DELTA / THRESHOLD),
                                    scalar2=None,
                                    op0=mybir.AluOpType.mult)
            nc.vector.tensor_tensor(out=af[s_], in0=mg[s_], in1=q[s_],
                                    op=mybir.AluOpType.is_lt)
            nc.gpsimd.dma_start(out=wout.ap()[s_], in_=wo[s_])
            nc.gpsimd.dma_start(out=amb.ap()[s_], in_=af[s_])

        pt = None
        for g in range(G):
            fs = loads.tile([P, GROUP_E], f16, tag="fs")
            fd = loads.tile([P, GROUP_E], f16, tag="fd")
            nc.sync.dma_start(out=fs[:], in_=us.ap()[g])
            nc.scalar.dma_start(out=fd[:], in_=ud.ap()[g])
            prod = scr.tile([P, GROUP_E], f16, tag="prod")
            nc.vector.tensor_mul(out=prod[:], in0=fs[:], in1=fd[:])
            for i in range(M):
                t = g * M + i
                j = t % PBLK
                if j == 0:
                    pt = psum.tile([P, PBLK], f32, tag="pt")
                nc.tensor.matmul(out=pt[:, j:j + 1],
                                 lhsT=prod[:, i * P:(i + 1) * P],
                                 rhs=ones[:], start=True, stop=True)
                if j == PBLK - 1 or t == T - 1:
                    blk = t // PBLK
                    nc.vector.tensor_copy(
                        out=inner[:, blk * PBLK:blk * PBLK + j + 1],
                        in_=pt[:, 0:j + 1])
                    finals(blk * PBLK, blk * PBLK + j + 1)
    nc.compile()
    return nc


def _build_rescue_nc():
    """NEFF3: exact fp32 recompute of flagged edges (2048 per core)."""
    nc = bacc.Bacc("TRN2", target_bir_lowering=False, debug=False,
                   num_devices=N_CORES)
    f32, i32 = mybir.dt.float32, mybir.dt.int32
    NCH = MR // MRC
    fa = nc.dram_tensor("fa", [NCH, P, MRC, D], f32, kind="ExternalInput")
    fb = nc.dram_tensor("fb", [NCH, P, MRC, D], f32, kind="ExternalInput")
    sclf = nc.dram_tensor("sclf", [P, 3, MR], f32, kind="ExternalInput")
    scli = nc.dram_tensor("scli", [P, 2, MR], i32, kind="ExternalInput")
    wro = nc.dram_tensor("wro", [P, MR], f32, kind="ExternalOutput")
    with tile.TileContext(nc) as tc, ExitStack() as ctx:
        mats = ctx.enter_context(tc.tile_pool(name="mats", bufs=1))
        loads = ctx.enter_context(tc.tile_pool(name="loads", bufs=2))
        scr = ctx.enter_context(tc.tile_pool(name="scr", bufs=2))
        sf = mats.tile([P, 3, MR], f32)
        si = mats.tile([P, 2, MR], i32)
        w_s, ns_s, nd_s = sf[:, 0, :], sf[:, 1, :], sf[:, 2, :]
        s_s, d_s = si[:, 0, :], si[:, 1, :]
        inner = mats.tile([P, MR], f32)
        nc.gpsimd.dma_start(out=sf[:], in_=sclf.ap())
        nc.gpsimd.dma_start(out=si[:], in_=scli.ap())
        for c in range(NCH):
            xa = loads.tile([P, MRC, D], f32, tag="xa")
            xb = loads.tile([P, MRC, D], f32, tag="xb")
            nc.sync.dma_start(out=xa[:], in_=fa.ap()[c])
            nc.scalar.dma_start(out=xb[:], in_=fb.ap()[c])
            prod = scr.tile([P, MRC, D], f32, tag="prod")
            nc.vector.tensor_mul(out=prod[:], in0=xa[:], in1=xb[:])
            nc.vector.tensor_reduce(out=inner[:, c * MRC:(c + 1) * MRC],
                                    in_=prod[:],
                                    axis=mybir.AxisListType.X,
                                    op=mybir.AluOpType.add)
        q = mats.tile([P, MR], f32)
        keep = mats.tile([P, MR], f32)
        eq = mats.tile([P, MR], f32)
        wo = mats.tile([P, MR], f32)
        nc.vector.tensor_mul(out=q[:], in0=ns_s, in1=nd_s)
        nc.vector.tensor_scalar(out=q[:], in0=q[:],
                                scalar1=float(EPS), scalar2=float(THRESHOLD),
                                op0=mybir.AluOpType.add,
                                op1=mybir.AluOpType.mult)
        nc.vector.tensor_tensor(out=keep[:], in0=inner[:], in1=q[:],
                                op=mybir.AluOpType.is_ge)
        nc.vector.tensor_tensor(out=eq[:], in0=s_s, in1=d_s,
                                op=mybir.AluOpType.is_equal)
        nc.vector.tensor_scalar(out=eq[:], in0=eq[:],
                                scalar1=1.0, scalar2=None,
                                op0=mybir.AluOpType.add)
        nc.vector.tensor_mul(out=wo[:], in0=w_s, in1=keep[:])
        nc.vector.tensor_mul(out=wo[:], in0=wo[:], in1=eq[:])
        nc.sync.dma_start(out=wro.ap(), in_=wo[:])
    nc.compile()
    return nc


def _get(name, builder):
    if name not in _cache:
        _cache[name] = builder()
    return _cache[name]


def _swz1_idx():
    """[NG1, P, G1] row indices (within a 12500-row shard) for NEFF1 layout."""
    if "swz1" not in _cache:
        g, p, m = np.meshgrid(np.arange(NG1), np.arange(P), np.arange(G1),
                              indexing="ij")
        t = g * G1 + m
        row = np.where(t < NT - 1, t * P + p, LAST_ROW0 + p)
        _cache["swz1"] = row.astype(np.int64)
    return _cache["swz1"]





def _rescue_perm():
    """[NCH, P, MRC] edge-slot indices for the NEFF3 [NCH,P,MRC,D] layout."""
    if "rperm" not in _cache:
        NCH = MR // MRC
        c, p, m = np.meshgrid(np.arange(NCH), np.arange(P), np.arange(MRC),
                              indexing="ij")
        _cache["rperm"] = ((c * MRC + m) * P + p).astype(np.int64)
    return _cache["rperm"]


def kernel(edge_index, edge_weight, features, _timing=None):
    edge_index = np.asarray(edge_index)
    edge_weight = np.asarray(edge_weight, dtype=np.float32)
    features = np.ascontiguousarray(np.asarray(features, dtype=np.float32))
    assert edge_index.shape == (2, N_EDGES) and features.shape == (N_NODES, D)

    src_all = edge_index[0].astype(np.int64)
    dst_all = edge_index[1].astype(np.int64)

    # symmetric-duplicate detection (host-side comparison only)
    half = N_EDGES // 2
    symmetric = (
        np.array_equal(src_all[:half], dst_all[half:])
        and np.array_equal(dst_all[:half], src_all[half:])
        and np.array_equal(edge_weight[:half], edge_weight[half:]))
    n_compute = half if symmetric else N_EDGES
    src, dst, w_all = src_all[:n_compute], dst_all[:n_compute], \
        edge_weight[:n_compute]

    results = []

    # ---- NEFF1: fp32 norms + fp16 table, row-sharded 8 ways ----
    nc1 = _get("norm", _build_norm_nc)
    swz1 = _swz1_idx()
    in_maps1 = [{"feat_sw":
                 features[k * NPC:(k + 1) * NPC][swz1]}
                for k in range(N_CORES)]
    res1 = run_bass_kernel_spmd(nc1, in_maps1, core_ids=list(range(N_CORES)),
                                **(_timing or {}))
    results.append(res1)
    u16_table = np.empty((N_NODES, D), dtype=np.float16)
    norm_full = np.empty(N_NODES, dtype=np.float32)
    norm16_full = np.empty(N_NODES, dtype=np.float16)
    swz1_flat = swz1.reshape(-1)
    for k in range(N_CORES):
        u16_table[k * NPC + swz1_flat] = \
            res1.results[k]["u16_sw"].reshape(-1, D)
        nrm = res1.results[k]["norm98"]             # [P, NT]
        norm_full[k * NPC + swz1_flat] = \
            nrm.T.reshape(NG1, G1, P).transpose(0, 2, 1).reshape(-1)
        nrm16 = res1.results[k]["norm98h"]
        norm16_full[k * NPC + swz1_flat] = \
            nrm16.T.reshape(NG1, G1, P).transpose(0, 2, 1).reshape(-1)

    # ---- NEFF2: per-edge fp16 inner products, threshold, ambiguity ----
    epc = n_compute // N_CORES
    T, SLOTS = _edge_geometry(epc)
    G = T // M
    nc2 = _get(f"edge{epc}", lambda: _build_edge_nc(epc))
    u16_T = np.ascontiguousarray(u16_table.T)       # [D, N] fp16
    in_maps2 = []
    for k in range(N_CORES):
        lo = k * epc
        s = np.zeros(SLOTS, dtype=np.int64)
        d = np.zeros(SLOTS, dtype=np.int64)
        w = np.zeros(SLOTS, dtype=np.float32)
        s[:epc] = src[lo:lo + epc]
        d[:epc] = dst[lo:lo + epc]
        w[:epc] = w_all[lo:lo + epc]
        in_maps2.append({
            # [G, 128(dim), 4096(edge)] fp16, transposed-gather layout
            "us": u16_T[:, s].reshape(P, G, GROUP_E).transpose(1, 0, 2).copy(),
            "ud": u16_T[:, d].reshape(P, G, GROUP_E).transpose(1, 0, 2).copy(),
            "w_m": w.reshape(T, P).T.copy(),
            "ns_m": norm16_full[s].reshape(T, P).T.copy(),
            "nd_m": norm16_full[d].reshape(T, P).T.copy(),
            "src_m": s.astype(np.int32).reshape(T, P).T.copy(),
            "dst_m": d.astype(np.int32).reshape(T, P).T.copy(),
        })
    res2 = run_bass_kernel_spmd(nc2, in_maps2, core_ids=list(range(N_CORES)),
                                **(_timing or {}))
    results.append(res2)

    out = np.empty(N_EDGES, dtype=edge_weight.dtype)
    amb = np.empty(n_compute, dtype=np.float32)
    for k in range(N_CORES):
        wo = res2.results[k]["wout"]                # [128, T]
        af = res2.results[k]["amb"]
        out[k * epc:(k + 1) * epc] = wo.T.ravel()[:epc]
        amb[k * epc:(k + 1) * epc] = af.T.ravel()[:epc]

    # ---- NEFF3: exact fp32 rescue of ambiguous edges ----
    amb_idx = np.flatnonzero(amb)
    if amb_idx.size:
        nc3 = _get("rescue", _build_rescue_nc)
        rperm = _rescue_perm()
        for c0 in range(0, amb_idx.size, R_TOTAL):
            chunk = amb_idx[c0:c0 + R_TOTAL]
            sa = np.zeros(R_TOTAL, dtype=np.int64)
            da = np.zeros(R_TOTAL, dtype=np.int64)
            wa = np.zeros(R_TOTAL, dtype=np.float32)
            sa[:chunk.size] = src[chunk]
            da[:chunk.size] = dst[chunk]
            wa[:chunk.size] = w_all[chunk]
            in_maps3 = []
            for k in range(N_CORES):
                lo = k * RSLOTS
                ssl = sa[lo:lo + RSLOTS]
                dsl = da[lo:lo + RSLOTS]
                wsl = wa[lo:lo + RSLOTS]
                sclf = np.stack([wsl.reshape(MR, P).T,
                                 norm_full[ssl].reshape(MR, P).T,
                                 norm_full[dsl].reshape(MR, P).T],
                                axis=1).copy()       # [P, 3, MR]
                scli = np.stack([ssl.astype(np.int32).reshape(MR, P).T,
                                 dsl.astype(np.int32).reshape(MR, P).T],
                                axis=1).copy()       # [P, 2, MR]
                in_maps3.append({
                    "fa": features[ssl[rperm]],     # [NCH, P, MRC, D] fp32
                    "fb": features[dsl[rperm]],
                    "sclf": sclf,
                    "scli": scli,
                })
            res3 = run_bass_kernel_spmd(nc3, in_maps3,
                                        core_ids=list(range(N_CORES)),
                                        **(_timing or {}))
            results.append(res3)
            fixed = np.concatenate(
                [res3.results[k]["wro"].T.ravel() for k in range(N_CORES)])
            out[chunk] = fixed[:chunk.size]

    if symmetric:
        out[half:] = out[:half]
    if _timing is not None:
        kernel._last = tuple(results)
    return out


# revision 25
# speedup vs baseline: 1.6665x; 1.6665x over previous
"""Trainium2 Bass kernel for Jaccard cosine-similarity edge masking.

out[e] = edge_weight[e] * (sim(e) >= 0.01) * (1 + (src==dst)),
sim(e) = <f_src, f_dst> / (||f_src|| * ||f_dst|| + 1e-8)

Three-stage device pipeline, edges sharded across 8 NeuronCores:

  NEFF1 (norm):   node table row-sharded 8 ways; each core computes
                  ||f|| per row (squares on the ACT engine, reduce on
                  DVE, all fp32) and emits an fp16 copy of its feature
                  shard via cast-during-DMA (SWDGE), plus fp32 and fp16
                  norm vectors.
  NEFF2 (edge):   per-edge inner products over host-gathered fp16 rows
                  (gather is pure indexing), streamed as ~1MiB linear
                  DMAs in a transposed [dim=partition, edge=free]
                  layout.  DVE does ONE fp16 multiply pass; the
                  per-edge reduction runs on the idle TensorEngine:
                  each 128-edge product tile is the matmul stationary
                  against a ones[128,1] moving vector, writing one PSUM
                  column of 128 distinct fp32 sums (~38ns/matmul); 512
                  matmuls fill a PSUM bank, drained by one DVE copy
                  straight into the edge-major [P,T] inner matrix.
                  Threshold test against q = thr*(ns*nd+eps) with
                  device-computed norms (fp16 stream); emits wout and
                  an ambiguity flag |inner - q| < q*(DELTA/thr),
                  finalized per 512-column block to overlap the tail.
  NEFF3 (rescue): flagged edges (~0.7%) recomputed exactly in fp32
                  from the original rows + device fp32 norms, making
                  the final output match the fp32 reference exactly
                  (fp16 noise is <~1.1e-4 in sim units, DELTA=8e-4).

If the edge list is detected (host-side comparison only) to be the
symmetric duplication [[s,d],[d,s]] with tied weights, only the first
half is computed and mirrored.

Host-side work is strictly indexing/layout: gathers of device-produced
tables, reshapes, and np.flatnonzero on a device-produced flag.  (This
environment's neuronxcc lowering miscompiles descriptor-based device
gather primitives, so row gathers are host-side.)
"""

import numpy as np
from contextlib import ExitStack

import concourse.bass as bass
import concourse.tile as tile
from concourse import bacc, mybir
from concourse.bass_utils import run_bass_kernel_spmd

N_NODES = 100000
N_EDGES = 1600000
D = 128
P = 128
N_CORES = 8
THRESHOLD = 0.01
EPS = 1e-8
DELTA = 8e-4          # ambiguity window in sim units (~3.5x max fp16 noise)

# NEFF1 geometry: 12500-row shard -> 98 tiles of 128 rows (last overlaps)
NPC = N_NODES // N_CORES          # 12500
NT = (NPC + P - 1) // P           # 98
LAST_ROW0 = NPC - P               # 12372
G1 = 14                           # tiles per load group
NG1 = NT // G1                    # 7

# NEFF2 geometry
M = 32                            # 128-edge tiles per load group (1 MiB fp16 DMA)

# NEFF3 geometry
MR = 16                           # rescue tiles (2048 edges/core)
MRC = 8                           # tiles per pipelined chunk
RSLOTS = MR * P                   # 2048
R_TOTAL = RSLOTS * N_CORES        # 16384

_cache = {}


def _build_norm_nc():
    """NEFF1: fp32 norms + fp16 table copy of a 12500-row shard."""
    nc = bacc.Bacc("TRN2", target_bir_lowering=False, debug=False,
                   num_devices=N_CORES)
    f32, f16 = mybir.dt.float32, mybir.dt.float16
    feat = nc.dram_tensor("feat_sw", [NG1, P, G1, D], f32, kind="ExternalInput")
    u16 = nc.dram_tensor("u16_sw", [NG1, P, G1, D], f16, kind="ExternalOutput")
    norm = nc.dram_tensor("norm98", [P, NT], f32, kind="ExternalOutput")
    norm16 = nc.dram_tensor("norm98h", [P, NT], f16, kind="ExternalOutput")
    with tile.TileContext(nc) as tc, ExitStack() as ctx:
        loads = ctx.enter_context(tc.tile_pool(name="loads", bufs=5))
        scr = ctx.enter_context(tc.tile_pool(name="scr", bufs=3))
        acc = ctx.enter_context(tc.tile_pool(name="acc", bufs=1))
        ss = acc.tile([P, NT], f32)
        for g in range(NG1):
            x = loads.tile([P, G1, D], f32, tag="x")
            eng = nc.sync if g % 2 == 0 else nc.scalar
            eng.dma_start(out=x[:], in_=feat.ap()[g])
            prod = scr.tile([P, G1, D], f32, tag="prod")
            # squares on the ACT engine (keeps DVE free for the reduce)
            nc.scalar.square(out=prod[:], in_=x[:])
            nc.vector.tensor_reduce(out=ss[:, g * G1:(g + 1) * G1],
                                    in_=prod[:],
                                    axis=mybir.AxisListType.X,
                                    op=mybir.AluOpType.add)
            # fp32 -> fp16 cast during DMA (SWDGE)
            nc.gpsimd.dma_start(out=u16.ap()[g], in_=x[:])
        nrm = acc.tile([P, NT], f32)
        nrm16 = acc.tile([P, NT], f16)
        nc.scalar.sqrt(out=nrm[:], in_=ss[:])
        nc.vector.tensor_copy(out=nrm16[:], in_=nrm[:])
        nc.sync.dma_start(out=norm.ap(), in_=nrm[:])
        nc.sync.dma_start(out=norm16.ap(), in_=nrm16[:])
    nc.compile()
    return nc


def _edge_geometry(epc):
    t = ((epc + P - 1) // P + M - 1) // M * M
    return t, t * P


GROUP_E = M * P                   # 4096 edges per load group
PBLK = 512                        # PSUM bank columns (f32)


def _build_edge_nc(epc):
    """NEFF2: fp16 products on DVE (transposed layout: partition dim =
    feature dim), per-128-edge sums via TensorE matmul with the edge tile
    as stationary and a ones[128,1] moving vector -> one PSUM column of
    128 distinct per-edge fp32 sums.  512 matmuls fill a [128,512] PSUM
    bank, drained with one DVE copy into the edge-major [P,T] inner
    matrix.  Threshold mask + ambiguity flag as before."""
    T, SLOTS = _edge_geometry(epc)
    G = T // M
    nc = bacc.Bacc("TRN2", target_bir_lowering=False, debug=False,
                   num_devices=N_CORES)
    f32, f16, i32 = mybir.dt.float32, mybir.dt.float16, mybir.dt.int32
    us = nc.dram_tensor("us", [G, P, GROUP_E], f16, kind="ExternalInput")
    ud = nc.dram_tensor("ud", [G, P, GROUP_E], f16, kind="ExternalInput")
    w_m = nc.dram_tensor("w_m", [P, T], f32, kind="ExternalInput")
    ns_m = nc.dram_tensor("ns_m", [P, T], f16, kind="ExternalInput")
    nd_m = nc.dram_tensor("nd_m", [P, T], f16, kind="ExternalInput")
    src_m = nc.dram_tensor("src_m", [P, T], i32, kind="ExternalInput")
    dst_m = nc.dram_tensor("dst_m", [P, T], i32, kind="ExternalInput")
    wout = nc.dram_tensor("wout", [P, T], f32, kind="ExternalOutput")
    amb = nc.dram_tensor("amb", [P, T], f16, kind="ExternalOutput")

    with tile.TileContext(nc) as tc, ExitStack() as ctx:
        mats = ctx.enter_context(tc.tile_pool(name="mats", bufs=1))
        loads = ctx.enter_context(tc.tile_pool(name="loads", bufs=6))
        scr = ctx.enter_context(tc.tile_pool(name="scr", bufs=4))
        psum = ctx.enter_context(tc.psum_pool(name="ps", bufs=2))

        w_s = mats.tile([P, T], f32)
        ns_s = mats.tile([P, T], f16)
        nd_s = mats.tile([P, T], f16)
        src_s = mats.tile([P, T], i32)
        dst_s = mats.tile([P, T], i32)
        inner = mats.tile([P, T], f32)
        ones = mats.tile([P, 1], f16)
        nc.vector.memset(ones[:], 1.0)
        nc.gpsimd.dma_start(out=w_s[:], in_=w_m.ap())
        nc.gpsimd.dma_start(out=ns_s[:], in_=ns_m.ap())
        nc.gpsimd.dma_start(out=nd_s[:], in_=nd_m.ap())
        nc.gpsimd.dma_start(out=src_s[:], in_=src_m.ap())
        nc.gpsimd.dma_start(out=dst_s[:], in_=dst_m.ap())

        q = mats.tile([P, T], f32)
        keep = mats.tile([P, T], f32)
        eq = mats.tile([P, T], f32)
        wo = mats.tile([P, T], f32)
        mg = mats.tile([P, T], f32)
        af = mats.tile([P, T], f16)

        def finals(c0, c1):
            """Threshold mask + ambiguity flag + output DMA for columns
            [c0, c1) -- called per drained PSUM block so the tail work
            overlaps the remaining stream."""
            s_ = (slice(None), slice(c0, c1))
            nc.vector.tensor_mul(out=q[s_], in0=ns_s[s_], in1=nd_s[s_])
            nc.vector.tensor_scalar(out=q[s_], in0=q[s_],
                                    scalar1=float(EPS),
                                    scalar2=float(THRESHOLD),
                                    op0=mybir.AluOpType.add,
                                    op1=mybir.AluOpType.mult)
            nc.vector.tensor_tensor(out=keep[s_], in0=inner[s_], in1=q[s_],
                                    op=mybir.AluOpType.is_ge)
            nc.vector.tensor_tensor(out=eq[s_], in0=src_s[s_], in1=dst_s[s_],
                                    op=mybir.AluOpType.is_equal)
            nc.vector.tensor_scalar(out=eq[s_], in0=eq[s_],
                                    scalar1=1.0, scalar2=None,
                                    op0=mybir.AluOpType.add)
            nc.vector.tensor_mul(out=wo[s_], in0=w_s[s_], in1=keep[s_])
            nc.vector.tensor_mul(out=wo[s_], in0=wo[s_], in1=eq[s_])
            # |inner - q| < q*(DELTA/thr)  ->  rescue flag
            nc.vector.tensor_tensor(out=mg[s_], in0=inner[s_], in1=q[s_],
                                    op=mybir.AluOpType.subtract)
            nc.scalar.activation(out=mg[s_], in_=mg[s_],
                                 func=mybir.ActivationFunctionType.Abs)
            nc.vector.tensor_scalar(out=q[s_], in0=q[s_],
                                    scalar1=float(DELTA / THRESHOLD),
                                    scalar2=None,
                                    op0=mybir.AluOpType.mult)
            nc.vector.tensor_tensor(out=af[s_], in0=mg[s_], in1=q[s_],
                                    op=mybir.AluOpType.is_lt)
            nc.gpsimd.dma_start(out=wout.ap()[s_], in_=wo[s_])
            nc.gpsimd.dma_start(out=amb.ap()[s_], in_=af[s_])

        pt = None
        for g in range(G):
            fs = loads.tile([P, GROUP_E], f16, tag="fs")
            fd = loads.tile([P, GROUP_E], f16, tag="fd")
            nc.sync.dma_start(out=fs[:], in_=us.ap()[g])
            nc.scalar.dma_start(out=fd[:], in_=ud.ap()[g])
            prod = scr.tile([P, GROUP_E], f16, tag="prod")
            nc.vector.tensor_mul(out=prod[:], in0=fs[:], in1=fd[:])
            for i in range(M):
                t = g * M + i
                j = t % PBLK
                if j == 0:
                    pt = psum.tile([P, PBLK], f32, tag="pt")
                nc.tensor.matmul(out=pt[:, j:j + 1],
                                 lhsT=prod[:, i * P:(i + 1) * P],
                                 rhs=ones[:], start=True, stop=True)
                if j == PBLK - 1 or t == T - 1:
                    blk = t // PBLK
                    nc.vector.tensor_copy(
                        out=inner[:, blk * PBLK:blk * PBLK + j + 1],
                        in_=pt[:, 0:j + 1])
                    finals(blk * PBLK, blk * PBLK + j + 1)
    nc.compile()
    return nc


def _build_rescue_nc():
    """NEFF3: exact fp32 recompute of flagged edges (2048 per core)."""
    nc = bacc.Bacc("TRN2", target_bir_lowering=False, debug=False,
                   num_devices=N_CORES)
    f32, i32 = mybir.dt.float32, mybir.dt.int32
    NCH = MR // MRC
    fa = nc.dram_tensor("fa", [NCH, P, MRC, D], f32, kind="ExternalInput")
    fb = nc.dram_tensor("fb", [NCH, P, MRC, D], f32, kind="ExternalInput")
    sclf = nc.dram_tensor("sclf", [P, 3, MR], f32, kind="ExternalInput")
    scli = nc.dram_tensor("scli", [P, 2, MR], i32, kind="ExternalInput")
    wro = nc.dram_tensor("wro", [P, MR], f32, kind="ExternalOutput")
    with tile.TileContext(nc) as tc, ExitStack() as ctx:
        mats = ctx.enter_context(tc.tile_pool(name="mats", bufs=1))
        loads = ctx.enter_context(tc.tile_pool(name="loads", bufs=2))
        scr = ctx.enter_context(tc.tile_pool(name="scr", bufs=2))
        sf = mats.tile([P, 3, MR], f32)
        si = mats.tile([P, 2, MR], i32)
        w_s, ns_s, nd_s = sf[:, 0, :], sf[:, 1, :], sf[:, 2, :]
        s_s, d_s = si[:, 0, :], si[:, 1, :]
        inner = mats.tile([P, MR], f32)
        nc.gpsimd.dma_start(out=sf[:], in_=sclf.ap())
        nc.gpsimd.dma_start(out=si[:], in_=scli.ap())
        for c in range(NCH):
            xa = loads.tile([P, MRC, D], f32, tag="xa")
            xb = loads.tile([P, MRC, D], f32, tag="xb")
            nc.sync.dma_start(out=xa[:], in_=fa.ap()[c])
            nc.scalar.dma_start(out=xb[:], in_=fb.ap()[c])
            prod = scr.tile([P, MRC, D], f32, tag="prod")
            nc.vector.tensor_mul(out=prod[:], in0=xa[:], in1=xb[:])
            nc.vector.tensor_reduce(out=inner[:, c * MRC:(c + 1) * MRC],
                                    in_=prod[:],
                                    axis=mybir.AxisListType.X,
                                    op=mybir.AluOpType.add)
        q = mats.tile([P, MR], f32)
        keep = mats.tile([P, MR], f32)
        eq = mats.tile([P, MR], f32)
        wo = mats.tile([P, MR], f32)
        nc.vector.tensor_mul(out=q[:], in0=ns_s, in1=nd_s)
        nc.vector.tensor_scalar(out=q[:], in0=q[:],
                                scalar1=float(EPS), scalar2=float(THRESHOLD),
                                op0=mybir.AluOpType.add,
                                op1=mybir.AluOpType.mult)
        nc.vector.tensor_tensor(out=keep[:], in0=inner[:], in1=q[:],
                                op=mybir.AluOpType.is_ge)
        nc.vector.tensor_tensor(out=eq[:], in0=s_s, in1=d_s,
                                op=mybir.AluOpType.is_equal)
        nc.vector.tensor_scalar(out=eq[:], in0=eq[:],
                                scalar1=1.0, scalar2=None,
                                op0=mybir.AluOpType.add)
        nc.vector.tensor_mul(out=wo[:], in0=w_s, in1=keep[:])
        nc.vector.tensor_mul(out=wo[:], in0=wo[:], in1=eq[:])
        nc.sync.dma_start(out=wro.ap(), in_=wo[:])
    nc.compile()
    return nc


def _get(name, builder):
    if name not in _cache:
        _cache[name] = builder()
    return _cache[name]


def _swz1_idx():
    """[NG1, P, G1] row indices (within a 12500-row shard) for NEFF1 layout."""
    if "swz1" not in _cache:
        g, p, m = np.meshgrid(np.arange(NG1), np.arange(P), np.arange(G1),
                              indexing="ij")
        t = g * G1 + m
        row = np.where(t < NT - 1, t * P + p, LAST_ROW0 + p)
        _cache["swz1"] = row.astype(np.int64)
    return _cache["swz1"]





def _rescue_perm():
    """[NCH, P, MRC] edge-slot indices for the NEFF3 [NCH,P,MRC,D] layout."""
    if "rperm" not in _cache:
        NCH = MR // MRC
        c, p, m = np.meshgrid(np.arange(NCH), np.arange(P), np.arange(MRC),
                              indexing="ij")
        _cache["rperm"] = ((c * MRC + m) * P + p).astype(np.int64)
    return _cache["rperm"]


def kernel(edge_index, edge_weight, features, _timing=None):
    edge_index = np.asarray(edge_index)
    edge_weight = np.asarray(edge_weight, dtype=np.float32)
    features = np.ascontiguousarray(np.asarray(features, dtype=np.float32))
    assert edge_index.shape == (2, N_EDGES) and features.shape == (N_NODES, D)

    src_all = edge_index[0].astype(np.int64)
    dst_all = edge_index[1].astype(np.int64)

    # symmetric-duplicate detection (host-side comparison only)
    half = N_EDGES // 2
    symmetric = (
        np.array_equal(src_all[:half], dst_all[half:])
        and np.array_equal(dst_all[:half], src_all[half:])
        and np.array_equal(edge_weight[:half], edge_weight[half:]))
    n_compute = half if symmetric else N_EDGES
    src, dst, w_all = src_all[:n_compute], dst_all[:n_compute], \
        edge_weight[:n_compute]

    results = []

    # ---- NEFF1: fp32 norms + fp16 table, row-sharded 8 ways ----
    nc1 = _get("norm", _build_norm_nc)
    swz1 = _swz1_idx()
    in_maps1 = [{"feat_sw":
                 features[k * NPC:(k + 1) * NPC][swz1]}
                for k in range(N_CORES)]
    res1 = run_bass_kernel_spmd(nc1, in_maps1, core_ids=list(range(N_CORES)),
                                **(_timing or {}))
    results.append(res1)
    u16_table = np.empty((N_NODES, D), dtype=np.float16)
    norm_full = np.empty(N_NODES, dtype=np.float32)
    norm16_full = np.empty(N_NODES, dtype=np.float16)
    swz1_flat = swz1.reshape(-1)
    for k in range(N_CORES):
        u16_table[k * NPC + swz1_flat] = \
            res1.results[k]["u16_sw"].reshape(-1, D)
        nrm = res1.results[k]["norm98"]             # [P, NT]
        norm_full[k * NPC + swz1_flat] = \
            nrm.T.reshape(NG1, G1, P).transpose(0, 2, 1).reshape(-1)
        nrm16 = res1.results[k]["norm98h"]
        norm16_full[k * NPC + swz1_flat] = \
            nrm16.T.reshape(NG1, G1, P).transpose(0, 2, 1).reshape(-1)

    # ---- NEFF2: per-edge fp16 inner products, threshold, ambiguity ----
    epc = n_compute // N_CORES
    T, SLOTS = _edge_geometry(epc)
    G = T // M
    nc2 = _get(f"edge{epc}", lambda: _build_edge_nc(epc))
    u16_T = np.ascontiguousarray(u16_table.T)       # [D, N] fp16
    in_maps2 = []
    for k in range(N_CORES):
        lo = k * epc
        s = np.zeros(SLOTS, dtype=np.int64)
        d = np.zeros(SLOTS, dtype=np.int64)
        w = np.zeros(SLOTS, dtype=np.float32)
        s[:epc] = src[lo:lo + epc]
        d[:epc] = dst[lo:lo + epc]
        w[:epc] = w_all[lo:lo + epc]
        in_maps2.append({
            # [G, 128(dim), 4096(edge)] fp16, transposed-gather layout
            "us": u16_T[:, s].reshape(P, G, GROUP_E).transpose(1, 0, 2).copy(),
            "ud": u16_T[:, d].reshape(P, G, GROUP_E).transpose(1, 0, 2).copy(),
            "w_m": w.reshape(T, P).T.copy(),
            "ns_m": norm16_full[s].reshape(T, P).T.copy(),
            "nd_m": norm16_full[d].reshape(T, P).T.copy(),
            "src_m": s.astype(np.int32).reshape(T, P).T.copy(),
            "dst_m": d.astype(np.int32).reshape(T, P).T.copy(),
        })
    res2 = run_bass_kernel_spmd(nc2, in_maps2, core_ids=list(range(N_CORES)),
                                **(_timing or {}))
    results.append(res2)

    out = np.empty(N_EDGES, dtype=edge_weight.dtype)
    amb = np.empty(n_compute, dtype=np.float32)
    for k in range(N_CORES):
        wo = res2.results[k]["wout"]                # [128, T]
        af = res2.results[k]["amb"]
        out[k * epc:(k + 1) * epc] = wo.T.ravel()[:epc]
        amb[k * epc:(k + 1) * epc] = af.T.ravel()[:epc]

    # ---- NEFF3: exact fp32 rescue of ambiguous edges ----
    amb_idx = np.flatnonzero(amb)
    if amb_idx.size:
        nc3 = _get("rescue", _build_rescue_nc)
        rperm = _rescue_perm()
        for c0 in range(0, amb_idx.size, R_TOTAL):
            chunk = amb_idx[c0:c0 + R_TOTAL]
            sa = np.zeros(R_TOTAL, dtype=np.int64)
            da = np.zeros(R_TOTAL, dtype=np.int64)
            wa = np.zeros(R_TOTAL, dtype=np.float32)
            sa[:chunk.size] = src[chunk]
            da[:chunk.size] = dst[chunk]
            wa[:chunk.size] = w_all[chunk]
            in_maps3 = []
            for k in range(N_CORES):
                lo = k * RSLOTS
                ssl = sa[lo:lo + RSLOTS]
                dsl = da[lo:lo + RSLOTS]
                wsl = wa[lo:lo + RSLOTS]
                sclf = np.stack([wsl.reshape(MR, P).T,
                                 norm_full[ssl].reshape(MR, P).T,
                                 norm_full[dsl].reshape(MR, P).T],
                                axis=1).copy()       # [P, 3, MR]
                scli = np.stack([ssl.astype(np.int32).reshape(MR, P).T,
                                 dsl.astype(np.int32).reshape(MR, P).T],
                                axis=1).copy()       # [P, 2, MR]
                in_maps3.append({
                    "fa": features[ssl[rperm]],     # [NCH, P, MRC, D] fp32
                    "fb": features[dsl[rperm]],
                    "sclf": sclf,
                    "scli": scli,
                })
            res3 = run_bass_kernel_spmd(nc3, in_maps3,
                                        core_ids=list(range(N_CORES)),
                                        **(_timing or {}))
            results.append(res3)
            fixed = np.concatenate(
                [res3.results[k]["wro"].T.ravel() for k in range(N_CORES)])
            out[chunk] = fixed[:chunk.size]

    if symmetric:
        out[half:] = out[:half]
    if _timing is not None:
        kernel._last = tuple(results)
    return out
